# revision 3
# baseline (speedup 1.0000x reference)
"""Trainium2 Bass kernel for LowRankMultiheadAttention (a2b path).

Sharding: 8 cores = batch(4) x head-half(2). Each core computes one batch
element and 8 of the 16 heads (one contiguous 512-wide half of embed_dim).

Per-core device program (all fp32, transposed layouts):
  kvT[din,t]  = x_b^T + pe^T  (prompt appended as key t=2048; softmax is
                permutation-invariant over keys so position doesn't matter)
  kT[e,t]     = Wk_half^T(as lhsT) @ kvT          (e on partitions)
  v[t,e]      = kvT(as lhsT) @ Wv_half^T, scaled by gates, stored per-head
                as [t, 64+1] with a ones column (fuses softmax denominator
                into the AV matmul)
  posS[e,50]  = Wpos_half^T @ pe[:50]^T           (tiny matmul)
  qT[d,q]     = debug^T + nearest-neighbor expansion of posS
                (host-known index map -> ~50 tensor_scalar_add segments)
  scoresT[t,q] per head, K=64 -> two heads packed via tile_position row
                tiling; exp on ACT with fused 1/8 scale (no max-subtract:
                scores ~ N(0,1), fp32 exp is safe)
  AV: psum[65,q] accumulated over t-chunks; PE-transpose -> [q,65];
      multiply rows by reciprocal of col 64 -> out[q, e_half]
"""

import os
import sys
from contextlib import ExitStack

import numpy as np

for _p in ("/opt/trn_rl_repo", "/root/.axon_site/_ro/trn_rl_repo"):
    if os.path.isdir(_p) and _p not in sys.path:
        sys.path.insert(0, _p)

import concourse.bass as bass  # noqa: E402
import concourse.mybir as mybir  # noqa: E402
import concourse.tile as tile  # noqa: E402
from concourse import bacc  # noqa: E402
from concourse.bass_utils import run_bass_kernel_spmd  # noqa: E402
from concourse.masks import make_identity  # noqa: E402

f32 = mybir.dt.float32
EXP = mybir.ActivationFunctionType.Exp

# Problem constants (hardcoded from the problem spec).
B, H, Q, HD = 4, 16, 801, 64
E = DIN = 1024
TB = 2048
T = TB + 1  # keys = x_b positions + prompt
EH = 512  # embed half per core
NHC = 8  # heads per core
NPAIR = 4  # head pairs per core
FB = 50  # full_b_step
N_DIN = DIN // 128  # 8 contraction chunks
N_EC = EH // 128  # 4 e-chunks of the half
NT = (T + 127) // 128  # 17 t-chunks (last has size 1)
KT_CH = [512, 512, 512, 512, 1]  # K-projection N (t) chunks
QC = [(0, 512), (512, Q - 512)]  # q chunks (N for scores / AV)
N_CORES = 8

_cache: dict = {}


def _segments(fa: int, fb: int):
    """Nearest-neighbor upsample index runs: [(src_col, start, end), ...]."""
    idx = (np.arange(fa) * fb) // fa
    segs = []
    s = 0
    for i in range(1, fa + 1):
        if i == fa or idx[i] != idx[s]:
            segs.append((int(idx[s]), s, i))
            s = i
    return segs


def _build(gates_val: float, segs) -> bacc.Bacc:
    nc = bacc.Bacc("TRN2", target_bir_lowering=False, debug=False,
                   num_devices=N_CORES)

    xbt = nc.dram_tensor("xbt", [DIN, TB], f32, kind="ExternalInput")
    pet = nc.dram_tensor("pet", [DIN, TB], f32, kind="ExternalInput")
    pmt = nc.dram_tensor("pmt", [DIN, 1], f32, kind="ExternalInput")
    dbt = nc.dram_tensor("dbt", [EH, Q], f32, kind="ExternalInput")
    wkt = nc.dram_tensor("wkt", [DIN, EH], f32, kind="ExternalInput")
    wvt = nc.dram_tensor("wvt", [DIN, EH], f32, kind="ExternalInput")
    wpt = nc.dram_tensor("wpt", [DIN, EH], f32, kind="ExternalInput")
    outd = nc.dram_tensor("out", [Q, EH], f32, kind="ExternalOutput")

    with tile.TileContext(nc) as tc, ExitStack() as top:
        const = top.enter_context(tc.tile_pool(name="const", bufs=1))
        idn = const.tile([128, 128], f32, tag="idn", name="idn")
        make_identity(nc, idn)
        posS = [const.tile([128, FB], f32, tag=f"posS{c}", name=f"posS{c}")
                for c in range(N_EC)]

        big = top.enter_context(tc.tile_pool(name="big", bufs=1))
        kT = [big.tile([128, T], f32, tag=f"kT{c}", name=f"kT{c}")
              for c in range(N_EC)]
        vaug = [big.tile([128, NHC * (HD + 1)], f32, tag=f"va{t}",
                         name=f"va{t}") for t in range(NT)]

        with tc.tile_pool(name="kvpool", bufs=1) as kvp:
            kvt = [kvp.tile([128, T], f32, tag=f"kv{j}", name=f"kv{j}")
                   for j in range(N_DIN)]

            # ---- Phase A: build kvT, project pos table ----
            with tc.tile_pool(name="lda", bufs=3) as lda, \
                 tc.tile_pool(name="wpa", bufs=1) as wpa, \
                 tc.tile_pool(name="pps", bufs=2, space="PSUM") as pps:
                for j in range(N_DIN):
                    r = slice(j * 128, (j + 1) * 128)
                    nc.sync.dma_start(out=kvt[j][:, 0:TB], in_=xbt[r, :])
                    pe_t = lda.tile([128, TB], f32, tag="pe", name="pe_t")
                    nc.sync.dma_start(out=pe_t, in_=pet[r, :])
                    nc.vector.tensor_add(kvt[j][:, 0:TB], kvt[j][:, 0:TB], pe_t)
                    nc.sync.dma_start(out=kvt[j][:, TB:T], in_=pmt[r, :])

                wpt_sb, pei = [], []
                for j in range(N_DIN):
                    r = slice(j * 128, (j + 1) * 128)
                    w = wpa.tile([128, EH], f32, tag=f"wp{j}", name=f"wp{j}")
                    nc.sync.dma_start(out=w, in_=wpt[r, :])
                    wpt_sb.append(w)
                    p = wpa.tile([128, FB], f32, tag=f"pei{j}", name=f"pei{j}")
                    nc.sync.dma_start(out=p, in_=pet[r, 0:FB])
                    pei.append(p)
                for c in range(N_EC):
                    pp = pps.tile([128, FB], f32, tag="pos", name="pp")
                    for j in range(N_DIN):
                        nc.tensor.matmul(pp, lhsT=wpt_sb[j][:, c * 128:(c + 1) * 128],
                                         rhs=pei[j], start=(j == 0),
                                         stop=(j == N_DIN - 1))
                    nc.vector.tensor_copy(posS[c], pp)

            # ---- Phase B: K and V projections ----
            with tc.tile_pool(name="ldw2", bufs=1) as ldw2, \
                 tc.tile_pool(name="ppk", bufs=3, space="PSUM") as ppk, \
                 tc.tile_pool(name="ppv", bufs=3, space="PSUM") as ppv:
                wk_sb, wv_sb = [], []
                for j in range(N_DIN):
                    r = slice(j * 128, (j + 1) * 128)
                    wk = ldw2.tile([128, EH], f32, tag=f"wk{j}", name=f"wk{j}")
                    nc.sync.dma_start(out=wk, in_=wkt[r, :])
                    wk_sb.append(wk)
                    wv = ldw2.tile([128, EH], f32, tag=f"wv{j}", name=f"wv{j}")
                    nc.sync.dma_start(out=wv, in_=wvt[r, :])
                    wv_sb.append(wv)

                toff = 0
                for tn in KT_CH:
                    for c in range(N_EC):
                        pk = ppk.tile([128, tn], f32, tag="k", name="pk")
                        for j in range(N_DIN):
                            nc.tensor.matmul(
                                pk, lhsT=wk_sb[j][:, c * 128:(c + 1) * 128],
                                rhs=kvt[j][:, toff:toff + tn],
                                start=(j == 0), stop=(j == N_DIN - 1))
                        nc.vector.tensor_copy(kT[c][:, toff:toff + tn], pk)
                    toff += tn

                for t in range(NT):
                    toff = t * 128
                    tsz = min(128, T - toff)
                    pv = ppv.tile([128, EH], f32, tag="v", name="pv")
                    for j in range(N_DIN):
                        nc.tensor.matmul(pv[:tsz, :],
                                         lhsT=kvt[j][:, toff:toff + tsz],
                                         rhs=wv_sb[j], start=(j == 0),
                                         stop=(j == N_DIN - 1))
                    for h in range(NHC):
                        nc.vector.tensor_scalar_mul(
                            vaug[t][:tsz, h * 65:h * 65 + 64],
                            pv[:tsz, h * 64:(h + 1) * 64], gates_val)
                    nc.vector.memset(vaug[t][:tsz, 64::65], 1.0)

        # ---- Phase C/D: qT build + attention ----
        with tc.tile_pool(name="attn", bufs=1) as att, \
             tc.tile_pool(name="ldq", bufs=2) as ldq, \
             tc.tile_pool(name="expp", bufs=3) as expp, \
             tc.tile_pool(name="opool", bufs=2) as opool, \
             tc.tile_pool(name="psc", bufs=2, space="PSUM") as psc, \
             tc.tile_pool(name="pav", bufs=1, space="PSUM") as pav, \
             tc.tile_pool(name="ptr", bufs=2, space="PSUM") as ptr:
            qT = [att.tile([128, Q], f32, tag=f"qT{p}", name=f"qT{p}")
                  for p in range(NPAIR)]
            n_qg = (Q + 127) // 128
            ost = [att.tile([128, EH], f32, tag=f"ost{g}", name=f"ost{g}")
                   for g in range(n_qg)]

            for p in range(NPAIR):
                dt_t = ldq.tile([128, Q], f32, tag="db", name="dt_t")
                nc.sync.dma_start(out=dt_t, in_=dbt[p * 128:(p + 1) * 128, :])
                for (j, s, e) in segs:
                    nc.vector.tensor_scalar_add(qT[p][:, s:e], dt_t[:, s:e],
                                                posS[p][:, j:j + 1])

            for p in range(NPAIR):
                hA, hB = 2 * p, 2 * p + 1
                for (qoff, qn) in QC:
                    pavA = pav.tile([65, qn], f32, tag="avA", name="pavA")
                    pavB = pav.tile([65, qn], f32, tag="avB", name="pavB")
                    for t in range(NT):
                        toff = t * 128
                        tsz = min(128, T - toff)
                        sA = psc.tile([tsz, qn], f32, tag="scA", name="sA")
                        sB = psc.tile([tsz, qn], f32, tag="scB", name="sB")
                        nc.tensor.matmul(sA, lhsT=kT[p][0:64, toff:toff + tsz],
                                         rhs=qT[p][0:64, qoff:qoff + qn],
                                         start=True, stop=True,
                                         tile_position=(0, 0))
                        nc.tensor.matmul(sB, lhsT=kT[p][64:128, toff:toff + tsz],
                                         rhs=qT[p][64:128, qoff:qoff + qn],
                                         start=True, stop=True,
                                         tile_position=(64, 0))
                        eA = expp.tile([tsz, qn], f32, tag="eA", name="eA")
                        eB = expp.tile([tsz, qn], f32, tag="eB", name="eB")
                        nc.scalar.activation(eA, sA, EXP, scale=0.125)
                        nc.scalar.activation(eB, sB, EXP, scale=0.125)
                        nc.tensor.matmul(pavA,
                                         lhsT=vaug[t][:tsz, hA * 65:hA * 65 + 65],
                                         rhs=eA, start=(t == 0),
                                         stop=(t == NT - 1))
                        nc.tensor.matmul(pavB,
                                         lhsT=vaug[t][:tsz, hB * 65:hB * 65 + 65],
                                         rhs=eB, start=(t == 0),
                                         stop=(t == NT - 1))
                    for head, pavX in ((hA, pavA), (hB, pavB)):
                        oX = opool.tile([65, qn], f32, tag="o", name="oX")
                        nc.vector.tensor_copy(oX, pavX)
                        nsub = (qn + 127) // 128
                        for si in range(nsub):
                            soff = si * 128
                            ssz = min(128, qn - soff)
                            g = (qoff + soff) // 128
                            tr = ptr.tile([ssz, 65], f32, tag="tr", name="tr")
                            nc.tensor.transpose(tr, oX[:, soff:soff + ssz],
                                                idn[0:65, 0:65])
                            rec = opool.tile([ssz, 1], f32, tag="rec",
                                             name="rec")
                            nc.vector.reciprocal(rec, tr[:, 64:65])
                            nc.vector.tensor_scalar_mul(
                                ost[g][:ssz, head * 64:head * 64 + 64],
                                tr[:, 0:64], rec)

            for g in range(n_qg):
                qoff = g * 128
                qsz = min(128, Q - qoff)
                nc.sync.dma_start(out=outd[qoff:qoff + qsz, :],
                                  in_=ost[g][:qsz, :])

    nc.compile()
    return nc


def get_program(gates_val: float, fa: int, fb: int) -> bacc.Bacc:
    key = (round(gates_val, 12), fa, fb)
    if key not in _cache:
        _cache[key] = _build(gates_val, _segments(fa, fb))
    return _cache[key]


def make_in_maps(x_b, debug, Wk, Wv, Wpos, prompt, pe):
    peT = np.ascontiguousarray(pe[0, :TB].T)
    pmtv = np.ascontiguousarray(prompt.reshape(DIN, 1))
    WkT = Wk.T  # [din, e]
    WvT = Wv.T
    WpT = Wpos.T
    in_maps = []
    for c in range(N_CORES):
        b, hf = divmod(c, 2)
        cols = slice(hf * EH, (hf + 1) * EH)
        in_maps.append({
            "xbt": np.ascontiguousarray(x_b[b].T),
            "pet": peT,
            "pmt": pmtv,
            "dbt": np.ascontiguousarray(
                debug[b, hf * NHC:(hf + 1) * NHC].transpose(0, 2, 1)
            ).reshape(EH, Q),
            "wkt": np.ascontiguousarray(WkT[:, cols]),
            "wvt": np.ascontiguousarray(WvT[:, cols]),
            "wpt": np.ascontiguousarray(WpT[:, cols]),
        })
    return in_maps


def assemble(results):
    out = np.empty((B, Q, E), np.float32)
    for c in range(N_CORES):
        b, hf = divmod(c, 2)
        out[b, :, hf * EH:(hf + 1) * EH] = results[c]["out"]
    return out


def kernel(**inputs) -> np.ndarray:
    x_b = np.asarray(inputs["x_b"], np.float32)
    debug = np.asarray(inputs["debug"], np.float32)
    Wk = np.asarray(inputs["Wk"], np.float32)
    Wv = np.asarray(inputs["Wv"], np.float32)
    Wpos = np.asarray(inputs["Wpos"], np.float32)
    prompt = np.asarray(inputs["prompt"], np.float32)
    gates = np.asarray(inputs["gates"], np.float32)
    pe = np.asarray(inputs["pe"], np.float32)
    fb = int(inputs["full_b_step"])
    fa = int(inputs["full_a_step"])
    assert x_b.shape == (B, TB, DIN) and debug.shape == (B, H, Q, HD)
    assert fa == Q and fb == FB, (fa, fb)

    nc = get_program(float(gates.reshape(-1)[0]), fa, fb)
    in_maps = make_in_maps(x_b, debug, Wk, Wv, Wpos, prompt, pe)
    res = run_bass_kernel_spmd(nc, in_maps, list(range(N_CORES)))
    return assemble(res.results)
